# revision 3
# baseline (speedup 1.0000x reference)
"""Trainium2 Bass kernel for nn_BasicLSTM: fc0 -> 10x LSTM(768) -> fc1.

Strategy: data-parallel over the 512 windows across 8 cores (64 windows each).
All matmul operands in bf16 (f32 PSUM accumulation, f32 cell state).

All 9 logical inputs are packed host-side into ONE bf16 DRAM buffer per core
(per-execute dispatch overhead in this runtime scales with the number of
large argument buffers, ~0.85ms each; one packed buffer pays it once).

Device schedule (per core, fused):
  The LSTM recurrence serializes on PE (h_{t-1} @ w_hhT per step) with
  ACT/DVE gate/state tails in between; the input GEMM for the NEXT layer
  (gx = X^T.T @ w_ihT + bias, batched over all timesteps) is emitted in
  units interleaved between recurrence steps so PE stays busy during the
  tails:
    wave A (m-tiles 0,1 = steps 0..3) after rec steps 4..9 of layer l,
    wave B (m-tiles 2,3,4 = steps 4..9) at the layer boundary and after
    rec steps 0..3 of layer l+1.
  gx lives as a single (128, mt, g4) tile: partition p holds batch row
  p%64 of step 2m + p//64. Even steps are read at partition base 0; odd
  steps are partition-shifted one step ahead by a small SBUF->SBUF DMA
  into a (64, g4) staging tile (engines cannot shift partitions; DMA can).
  w_hhT and gx are double-buffered across layers; w_ihT streams as 512-wide
  chunks (re-fetched once per wave, ~2x weight traffic, still far under
  HBM roofline). PSUM->SBUF gx copies run on ACT; f*c on GPSIMD; the rest
  of the elementwise on DVE.
fc1 contracts X^T_last over all (t, h) with streamed fc1_w chunks.
"""
import numpy as np
import ml_dtypes

H = 768
G = 4 * H          # 3072
W = 10             # time steps (window size)
L = 10             # layers
B_FULL = 512
NCORES = 8
BL = B_FULL // NCORES  # 64 windows per core

# ---- packed-input layout (bf16 element offsets) ----
_SIZES = [
    ("xposT", H * W * BL),        # (H, W*BL)
    ("fc0wT", H * H),             # (H, H)
    ("fc0b", H),                  # (1, H)
    ("wihT", L * H * G),          # (L, H, G)
    ("whhT", L * H * G),          # (L, H, G)
    ("biasT", L * G),             # (L, 1, G)
    ("fc1wT", W * H * H),         # (W*H, H)
    ("fc1bT", H),                 # (1, H)
]
OFF = {}
_c = 0
for _n, _s in _SIZES:
    OFF[_n] = _c
    _c += _s
TOTAL_ELEMS = _c

_CACHE = {}


def build_program(h=H, w=W, nl=L, bl=BL):
    import concourse.mybir as mybir
    import concourse.tile as tile
    from concourse import bacc
    from concourse.masks import make_identity

    F32 = mybir.dt.float32
    BF16 = mybir.dt.bfloat16
    AF = mybir.ActivationFunctionType
    OP = mybir.AluOpType

    g4 = 4 * h
    kt = h // 128           # k-tiles over h
    nch = g4 // 512         # 512-wide chunks over the gate dim
    mt = (w * bl) // 128    # m-tiles over the (t, b) axis
    fn1 = h // 2            # fc1 output chunk (two psum chunks)
    assert h % 128 == 0 and g4 % 512 == 0 and (w * bl) % 128 == 0 and bl == 64

    nc = bacc.Bacc("TRN2", target_bir_lowering=False, debug=False)

    pk = nc.dram_tensor("packed", [TOTAL_ELEMS], BF16, kind="ExternalInput")
    out_d = nc.dram_tensor("out", [bl, h], F32, kind="ExternalOutput")

    def pv(name, n, pattern=None, **axes):
        ap = pk[OFF[name]:OFF[name] + n]
        return ap.rearrange(pattern, **axes) if pattern else ap

    xposT_v = pv("xposT", h * w * bl, "(k p c) -> p k c", p=128, c=w * bl)
    fc0wT_v = pv("fc0wT", h * h, "(k p ho) -> p k ho", p=128, ho=h)
    fc0b_v = pv("fc0b", h, "(a ho) -> a ho", a=1)
    fc1bT_v = pv("fc1bT", h, "(a ho) -> a ho", a=1)

    def wihT_v(l, k, js):
        base = OFF["wihT"] + l * h * g4
        ap = pk[base:base + h * g4].rearrange("(k p g) -> p k g", p=128, g=g4)
        return ap[:, k, js]

    def whhT_v(l):
        base = OFF["whhT"] + l * h * g4
        return pk[base:base + h * g4].rearrange("(k p g) -> p k g", p=128, g=g4)

    def biasT_v(l):
        base = OFF["biasT"] + l * g4
        return pk[base:base + g4].rearrange("(a g) -> a g", a=1)

    def fc1wT_v(ktile, ns):
        base = OFF["fc1wT"] + ktile * 128 * h
        ap = pk[base:base + 128 * h].rearrange("(r c) -> r c", c=h)
        return ap[:, ns]

    with tile.TileContext(nc) as tc, \
         tc.tile_pool(name="persist", bufs=1) as pp, \
         tc.tile_pool(name="whhp", bufs=2) as whhp, \
         tc.tile_pool(name="gxpool", bufs=2) as gxpool, \
         tc.tile_pool(name="gxsp", bufs=1) as gxsp, \
         tc.tile_pool(name="biasp", bufs=2) as biasp, \
         tc.tile_pool(name="wstream", bufs=6) as wsp, \
         tc.tile_pool(name="gpool", bufs=1) as gp, \
         tc.tile_pool(name="gatep", bufs=1) as gatep, \
         tc.tile_pool(name="tmp", bufs=1) as tp, \
         tc.tile_pool(name="cpool", bufs=2) as cp, \
         tc.tile_pool(name="hpool", bufs=2) as hp, \
         tc.tile_pool(name="psR", bufs=2, space="PSUM") as psR, \
         tc.tile_pool(name="psG", bufs=3, space="PSUM") as psG, \
         tc.tile_pool(name="psT", bufs=1, space="PSUM") as psT, \
         tc.tile_pool(name="psF", bufs=2, space="PSUM") as psF:

        # ---- persistent tiles ----
        XT = pp.tile([128, kt, w * bl], BF16)      # h^T / layer-input storage
        onesb = pp.tile([1, 512], BF16)
        nc.vector.memset(onesb[:], 1.0)
        idb = pp.tile([64, 64], BF16)
        make_identity(nc, idb[:])
        fc0b_sb = pp.tile([1, h], BF16)
        nc.sync.dma_start(fc0b_sb[:], fc0b_v)
        fc1b_sb = pp.tile([1, h], BF16)
        nc.sync.dma_start(fc1b_sb[:], fc1bT_v)

        def load_whh(l):
            t = whhp.tile([128, kt, g4], BF16, tag="whh", name=f"whh{l}")
            v = whhT_v(l)
            for k in range(kt):
                nc.sync.dma_start(t[:, k, :], v[:, k, :])
            return t

        def load_bias(l):
            t = biasp.tile([1, g4], BF16, tag="bias", name=f"bias{l}")
            nc.sync.dma_start(t[:], biasT_v(l))
            return t

        def emit_gx_unit(lsrc, ms, j, gx_dst, bias_sb):
            """gx_dst[:, m, js] = XT[:, :, m-cols].T @ wihT[lsrc][:, js] + bias."""
            js = slice(j * 512, (j + 1) * 512)
            chunks = []
            for k in range(kt):
                wk = wsp.tile([128, 512], BF16, tag="wch")
                nc.sync.dma_start(wk[:], wihT_v(lsrc, k, js))
                chunks.append(wk)
            for m in ms:
                ps = psG.tile([128, 512], F32, tag="gxw",
                              name=f"gxps_{lsrc}_{j}_{m}")
                for k in range(kt):
                    nc.tensor.matmul(
                        ps[:], XT[:, k, m * 128:(m + 1) * 128], chunks[k][:],
                        start=(k == 0), stop=False)
                nc.tensor.matmul(
                    ps[:], onesb[:, 0:128], bias_sb[:, js], start=False, stop=True)
                nc.scalar.activation(gx_dst[:, m, js], ps[:], AF.Copy)

        def rec_step(l, t, gx_t, whh_cur, c_cur):
            """One LSTM step; returns new cell tile. gx_t: (64, g4) bf16 AP."""
            if t == 0:
                gsrc = gx_t
            else:
                g = gp.tile([64, g4], BF16, tag="g", name=f"g_{l}_{t}")
                for j in range(nch):
                    js = slice(j * 512, (j + 1) * 512)
                    ps = psR.tile([128, 512], F32, tag="recps",
                                  name=f"recps_{l}_{t}_{j}")
                    for k in range(kt):
                        nc.tensor.matmul(
                            ps[0:64, :],
                            XT[:, k, (t - 1) * 64:t * 64],
                            whh_cur[:, k, js],
                            start=(k == 0), stop=(k == kt - 1))
                    nc.vector.tensor_tensor(
                        g[:, js], ps[0:64, :], gx_t[:, js], OP.add)
                gsrc = g
            gates = gatep.tile([64, g4], BF16, tag="gates", name=f"gates_{l}_{t}")
            nc.scalar.activation(gates[:, 0:2 * h], gsrc[:, 0:2 * h], AF.Sigmoid)
            nc.scalar.activation(gates[:, 2 * h:3 * h], gsrc[:, 2 * h:3 * h], AF.Tanh)
            nc.scalar.activation(gates[:, 3 * h:4 * h], gsrc[:, 3 * h:4 * h], AF.Sigmoid)
            c_new = cp.tile([64, h], F32, tag="c", name=f"c_{l}_{t}")
            if t == 0:
                nc.vector.tensor_tensor(
                    c_new[:], gates[:, 0:h], gates[:, 2 * h:3 * h], OP.mult)
            else:
                t1 = tp.tile([64, h], F32, tag="t1")
                nc.vector.tensor_tensor(
                    t1[:], gates[:, 0:h], gates[:, 2 * h:3 * h], OP.mult)
                t2 = tp.tile([64, h], F32, tag="t2")
                nc.gpsimd.tensor_tensor(t2[:], gates[:, h:2 * h], c_cur[:], OP.mult)
                nc.vector.tensor_tensor(c_new[:], t1[:], t2[:], OP.add)
            tc_t = tp.tile([64, h], BF16, tag="tc")
            nc.scalar.activation(tc_t[:], c_new[:], AF.Tanh)
            hh = hp.tile([64, h], BF16, tag="hh", name=f"hh_{l}_{t}")
            nc.vector.tensor_tensor(hh[:], gates[:, 3 * h:4 * h], tc_t[:], OP.mult)
            trp = psT.tile([128, kt * 64], BF16, tag="trp")
            for s in range(kt):
                nc.tensor.transpose(
                    trp[:, s * 64:(s + 1) * 64], hh[:, s * 128:(s + 1) * 128], idb[:])
            nc.vector.tensor_copy(
                XT[:, :, t * 64:(t + 1) * 64],
                trp[:].rearrange("p (s x) -> p s x", s=kt))
            return c_new

        # ---- fc0: XT <- fc0_wT.T @ xposT + fc0_b ----
        fc0w = whhp.tile([128, kt, h], BF16, tag="whh", name="fc0w")
        nc.sync.dma_start(fc0w[:], fc0wT_v)
        xpt = gxpool.tile([128, kt, w * bl], BF16, tag="gx", name="xpt")
        nc.sync.dma_start(xpt[:], xposT_v)
        fc0_chunks = [(c0, min(512, w * bl - c0)) for c0 in range(0, w * bl, 512)]
        for m in range(kt):
            for c0, cw in fc0_chunks:
                ps = psG.tile([128, 512], F32, tag="gxw", name=f"fc0ps_{m}_{c0}")
                for k in range(kt):
                    nc.tensor.matmul(
                        ps[:, :cw],
                        fc0w[:, k, m * 128:(m + 1) * 128],
                        xpt[:, k, c0:c0 + cw],
                        start=(k == 0), stop=False,
                    )
                nc.tensor.matmul(
                    ps[:, :cw], fc0b_sb[:, m * 128:(m + 1) * 128],
                    onesb[:, 0:cw], start=False, stop=True)
                nc.scalar.activation(XT[:, m, c0:c0 + cw], ps[:, :cw], AF.Copy)

        # ---- layer 0 prologue: whh0, bias0, full gx0 ----
        whh_cur = load_whh(0)
        bias_cur = load_bias(0)
        gx_cur = gxpool.tile([128, mt, g4], BF16, tag="gx", name="gx0")
        for j in range(nch):
            emit_gx_unit(0, range(mt), j, gx_cur, bias_cur)

        # ---- layers (rec fused with next layer's gx waves) ----
        pendB = None  # (gx tile, bias tile, layer) with units j=2..5 pending
        for l in range(nl):
            if l < nl - 1:
                bias_nx = load_bias(l + 1)
                gx_next = gxpool.tile([128, mt, g4], BF16, tag="gx",
                                      name=f"gx{l + 1}")
            c_cur = None
            gxs_next = None
            for t in range(w):
                if t % 2 == 0:
                    gx_t = gx_cur[0:64, t // 2, :]
                else:
                    gx_t = gxs_next[:]
                c_cur = rec_step(l, t, gx_t, whh_cur, c_cur)
                if t % 2 == 0:
                    # partition-shift next (odd) step's gx slice one step ahead
                    gxs_next = gxsp.tile([64, g4], BF16, tag="gxs",
                                         name=f"gxs_{l}_{t + 1}")
                    nc.sync.dma_start(gxs_next[:], gx_cur[64:128, t // 2, :])
                if pendB is not None and t <= 3:
                    gxB, biasB, lB = pendB
                    emit_gx_unit(lB, (2, 3, 4), 2 + t, gxB, biasB)
                    if t == 3:
                        pendB = None
                if l < nl - 1 and t >= 4:
                    emit_gx_unit(l + 1, (0, 1), t - 4, gx_next, bias_nx)
                if t == 2 and l < nl - 1:
                    whh_next = load_whh(l + 1)
            if l < nl - 1:
                emit_gx_unit(l + 1, (2, 3, 4), 0, gx_next, bias_nx)
                emit_gx_unit(l + 1, (2, 3, 4), 1, gx_next, bias_nx)
                pendB = (gx_next, bias_nx, l + 1)
                gx_cur, whh_cur, bias_cur = gx_next, whh_next, bias_nx

        # ---- fc1 ----
        out_sb = pp.tile([64, h], F32)
        for nchunk in range(2):
            ns = slice(nchunk * fn1, (nchunk + 1) * fn1)
            ps = psF.tile([128, 512], F32, tag="fc1acc", name=f"fc1ps_{nchunk}")
            for ktile in range(w * kt):
                t, s = ktile // kt, ktile % kt
                wk = wsp.tile([128, fn1], BF16, tag="fc1w")
                nc.sync.dma_start(wk[:], fc1wT_v(ktile, ns))
                nc.tensor.matmul(
                    ps[0:64, :fn1], XT[:, s, t * 64:(t + 1) * 64], wk[:],
                    start=(ktile == 0), stop=False)
            nc.tensor.matmul(
                ps[0:64, :fn1], onesb[:, 0:64], fc1b_sb[:, ns], start=False, stop=True)
            nc.vector.tensor_copy(out_sb[:, ns], ps[0:64, :fn1])
        nc.sync.dma_start(out_d[:], out_sb[:])

    nc.compile()
    return nc


def _pack_shared(inputs, h=H, w=W, nl=L):
    """Weights part of the packed buffer (identical across cores), bf16."""
    f32 = np.float32
    bf16 = ml_dtypes.bfloat16
    parts = [
        np.ascontiguousarray(np.asarray(inputs["fc0_w"], f32).T).astype(bf16).ravel(),
        np.asarray(inputs["fc0_b"], f32).astype(bf16).ravel(),
        np.ascontiguousarray(
            np.asarray(inputs["w_ih"], f32).transpose(0, 2, 1)).astype(bf16).ravel(),
        np.ascontiguousarray(
            np.asarray(inputs["w_hh"], f32).transpose(0, 2, 1)).astype(bf16).ravel(),
        (np.asarray(inputs["b_ih"], f32) + np.asarray(inputs["b_hh"], f32))
        .astype(bf16).ravel(),
        np.ascontiguousarray(np.asarray(inputs["fc1_w"], f32).T).astype(bf16).ravel(),
        np.asarray(inputs["fc1_b"], f32).astype(bf16).ravel(),
    ]
    return np.concatenate(parts)


def prep_inputs(inputs):
    """Build per-core packed input buffers."""
    bf16 = ml_dtypes.bfloat16
    shared = _pack_shared(inputs)
    x = np.ascontiguousarray(
        np.asarray(inputs["x_position"], np.float32)).reshape(-1, W, H)
    in_maps = []
    for c in range(NCORES):
        xc = x[c * BL:(c + 1) * BL]
        xposT = np.ascontiguousarray(
            xc.transpose(2, 1, 0).reshape(H, W * BL)).astype(bf16).ravel()
        packed = np.empty(TOTAL_ELEMS, bf16)
        packed[:xposT.size] = xposT
        packed[xposT.size:] = shared
        in_maps.append({"packed": packed})
    return in_maps


def run_on_cores(in_maps, trace=False, **kwargs):
    from concourse.bass_utils import run_bass_kernel_spmd
    if "nc" not in _CACHE:
        _CACHE["nc"] = build_program()
    nc = _CACHE["nc"]
    return run_bass_kernel_spmd(
        nc, in_maps, core_ids=list(range(NCORES)), trace=trace, **kwargs)


def kernel(**inputs) -> np.ndarray:
    in_maps = prep_inputs(inputs)
    res = run_on_cores(in_maps)
    outs = [res.results[c]["out"] for c in range(NCORES)]
    full = np.concatenate(outs, axis=0)            # (512, 768)
    return np.ascontiguousarray(full.reshape(-1, 3).astype(np.float32))
